# revision 7
# baseline (speedup 1.0000x reference)
"""Trainium2 Bass kernel for nn_Loss2D_57432302682561 (v2).

Math per view v (V = 40000 views, 68 landmarks each):
    y block  = points_y[68 + 68v : 68 + 68(v+1)]          # [68, 15]
    pt       = y[:, 0:2]                                   # target 2D points
    scale    = y[0, 2];  R = y[0, 3:12].reshape(3,3);  t = y[0, 12:15]
    M        = inv(scale * R) = adj(R) / (scale * det(R))  # [3, 3]
    proj     = (points_x - t) @ M  -> first 2 cols         # [68, 2]
    mask     = (pt[:,0] >= 0) | (pt[:,1] >= 0)
    dist     = sqrt(sum((pt - proj)^2, -1))
    loss_v   = sum(dist * mask) / sum(mask)
    out      = sum_v loss_v / V^2

v2 design (8 NeuronCores, data-parallel over views; DMA-roofline bound):
  - Chunks of 12 tiles (128 views each) double-buffered; one contiguous
    slab DMA per chunk (full-bandwidth 4080B descriptors).
  - Header math (3x3 inverse) via duplicated-row cross products:
    inv(R) cols 0/1 = cross(r1,r2), cross(r2,r0) over scale*det.  ~20 wide
    ops per chunk instead of 41 tiny ops (dup-copies on ACT, math on DVE).
  - Weights for ALL tiles of a chunk transposed in ONE PE transpose
    ([128 views, 128 weight-cols] -> PSUM), one PSUM->SBUF copy.  Weight
    cols are grouped 32 per 3-tile group (24 data + 8 pad) so matmul
    lhsT slices land on legal partition bases {0,32,64,96}.
  - Projection: one PE matmul per 3-tile group with a block-diagonal
    [24, 408] streamed matrix (replicated in all 4 SBUF quadrants so
    lhsT/rhs partition bases match).
  - Elementwise work grouped over 3 tiles per op (DVE); Square on ACT;
    per-view mask count via fused tensor_scalar accum (cheap DVE
    accumulator); per-view distance sum via per-tile ACT Sqrt accum.
  - Per-core output: num[128, nt] (masked dist sum), den[128, nt]
    (mask count); host computes num/den and the final reduction.
"""

import os
import sys

import numpy as np

for _p in ("/opt/trn_rl_repo",):
    if _p not in sys.path and os.path.isdir(_p):
        sys.path.insert(0, _p)

import concourse.bass as bass
import concourse.bacc as bacc
import concourse.tile as tile
from concourse import mybir
from concourse.bass_utils import run_bass_kernel_spmd
from concourse.masks import make_identity
from contextlib import ExitStack

F32 = mybir.dt.float32
BF16 = mybir.dt.bfloat16
NPTS = 68
ROWW = 15
VROW = NPTS * ROWW  # 1020 floats per view block
N_CORES = 8
V_TOTAL = 40000
V_CORE = V_TOTAL // N_CORES  # 5000
VPT = 128  # views per tile (partition dim)
B = 12     # tiles per chunk
GR = 3     # tiles per matmul group


def build_nc(v_core=V_CORE):
    nt = (v_core + VPT - 1) // VPT
    chunk_sizes = []
    t = 0
    while t < nt:
        sz = min(B, nt - t)
        chunk_sizes.append(sz)
        t += sz

    nc = bacc.Bacc()
    y = nc.dram_tensor("y", [v_core * NPTS, ROWW], F32, kind="ExternalInput")
    xq_d = nc.dram_tensor("xaug3", [128, GR * 2 * NPTS], BF16, kind="ExternalInput")
    num_o = nc.dram_tensor("num", [VPT, nt], F32, kind="ExternalOutput")
    den_o = nc.dram_tensor("den", [VPT, nt], F32, kind="ExternalOutput")

    # [v, (l c)] view of the input: one view block = 1020 contiguous floats
    y2 = y.rearrange("(v l) c -> v (l c)", l=NPTS)

    AF = mybir.ActivationFunctionType
    ALU = mybir.AluOpType

    with ExitStack() as ctx:
        tc = ctx.enter_context(tile.TileContext(nc))
        consts = ctx.enter_context(tc.tile_pool(name="consts", bufs=1))
        slabs = ctx.enter_context(tc.tile_pool(name="slabs", bufs=2))
        hdr = ctx.enter_context(tc.tile_pool(name="hdr", bufs=2))
        hvp = ctx.enter_context(tc.tile_pool(name="hvp", bufs=2))
        lhsp = ctx.enter_context(tc.tile_pool(name="lhsp", bufs=2))
        work = ctx.enter_context(tc.tile_pool(name="work", bufs=3))
        outp = ctx.enter_context(tc.tile_pool(name="outp", bufs=1))
        psum_p = ctx.enter_context(tc.tile_pool(name="psum_p", bufs=4, space="PSUM"))
        psum_t = ctx.enter_context(tc.tile_pool(name="psum_t", bufs=2, space="PSUM"))

        identity = consts.tile([128, 128], F32)
        make_identity(nc, identity)

        # Streamed projection matrix: block-diagonal [24, 408] (three [8,136]
        # xaug blocks), replicated in all four partition quadrants so the
        # matmul rhs partition base can match any lhsT group base.
        xq = consts.tile([128, GR * 2 * NPTS], BF16, name="xq")
        nc.sync.dma_start(out=xq, in_=xq_d[:, :])

        NUM = outp.tile([VPT, nt], F32)
        DEN = outp.tile([VPT, nt], F32)

        t0 = 0
        for bc in chunk_sizes:
            v0 = t0 * VPT
            n_views = min(v_core - v0, bc * VPT)
            nf = n_views // VPT  # full tiles
            rem = n_views - nf * VPT
            ng = (bc + GR - 1) // GR          # matmul groups this chunk
            ngf = bc // GR                    # full (3-tile) groups
            nf3 = ngf * GR                    # tiles covered by full groups
            gs_t = bc - nf3                   # tail group size (0..2)

            # Small early header DMA: 13 floats per view (scale, R, t).
            # Completes quickly, so header math + weight transpose for this
            # chunk overlap the previous chunk's bulk stream.
            hdrH = hdr.tile([VPT, B, 13], F32, tag="hdrH")
            if nf > 0:
                hsrc = y2[v0 : v0 + nf * VPT].rearrange(
                    "(w p) f -> p w f", p=VPT
                )[:, :, 2:15]
                nc.sync.dma_start(out=hdrH[:, 0:nf, :], in_=hsrc)
            if rem > 0:
                nc.sync.dma_start(
                    out=hdrH[0:rem, nf, :],
                    in_=y2[v0 + nf * VPT : v0 + n_views, 2:15],
                )
                # invalid partitions: real (replicated) headers keep all
                # lanes finite; host ignores their num/den entries
                nc.sync.dma_start(
                    out=hdrH[rem:VPT, nf, :], in_=y2[v0 : v0 + VPT - rem, 2:15]
                )

            # Bulk stream, one DMA per 3-tile group so elementwise compute
            # starts as soon as each group lands.
            slab = slabs.tile([VPT, B, VROW], F32, tag="slab")
            for g in range(ng):
                lw0 = g * GR
                gv0 = v0 + lw0 * VPT
                gnv = min(n_views - lw0 * VPT, GR * VPT)
                gnf = gnv // VPT
                grem = gnv - gnf * VPT
                if gnf > 0:
                    src = y2[gv0 : gv0 + gnf * VPT].rearrange(
                        "(w p) f -> p w f", p=VPT
                    )
                    nc.sync.dma_start(
                        out=slab[:, lw0 : lw0 + gnf, :], in_=src
                    )
                if grem > 0:
                    nc.sync.dma_start(
                        out=slab[0:grem, lw0 + gnf, :],
                        in_=y2[gv0 + gnf * VPT : gv0 + gnv],
                    )
                    nc.sync.dma_start(
                        out=slab[grem:VPT, lw0 + gnf, :],
                        in_=y2[v0 : v0 + VPT - grem],
                    )

            # ---- header math: M = inv(scale*R) cols 0,1 + bias rows ----
            # hv[:, 32g + 8t + k], k in 0..7: [M00,M10,M20,c0,M01,M11,M21,c1]
            hv = hvp.tile([VPT, 128], F32, tag="hv")
            nc.gpsimd.memset(hv, 0.0)

            # duplicated row buffers for cross products:
            # A = [r1 r1-dup | r2 r2-dup], B = [r2 r2-dup | r0 r0-dup]
            A = hdr.tile([VPT, B, 2, 5], F32, tag="A")
            Bt = hdr.tile([VPT, B, 2, 5], F32, tag="Bt")
            r12 = hdrH[:, 0:bc, 4:10].rearrange("p w (r c) -> p w r c", r=2)
            nc.scalar.copy(A[:, 0:bc, :, 0:3], r12)
            nc.scalar.copy(A[:, 0:bc, :, 3:5], r12[:, :, :, 0:2])
            nc.scalar.copy(Bt[:, 0:bc, 0, 0:3], hdrH[:, 0:bc, 7:10])
            nc.scalar.copy(Bt[:, 0:bc, 0, 3:5], hdrH[:, 0:bc, 7:9])
            nc.scalar.copy(Bt[:, 0:bc, 1, 0:3], hdrH[:, 0:bc, 1:4])
            nc.scalar.copy(Bt[:, 0:bc, 1, 3:5], hdrH[:, 0:bc, 1:3])

            # X[:, w, 0, :] = cross(r1, r2);  X[:, w, 1, :] = cross(r2, r0)
            P = hdr.tile([VPT, B, 2, 3], F32, tag="P")
            Q = hdr.tile([VPT, B, 2, 3], F32, tag="Q")
            X = hdr.tile([VPT, B, 2, 3], F32, tag="X")
            nc.vector.tensor_tensor(
                P[:, 0:bc], A[:, 0:bc, :, 1:4], Bt[:, 0:bc, :, 2:5], op=ALU.mult
            )
            nc.vector.tensor_tensor(
                Q[:, 0:bc], A[:, 0:bc, :, 2:5], Bt[:, 0:bc, :, 1:4], op=ALU.mult
            )
            nc.vector.tensor_tensor(
                X[:, 0:bc], P[:, 0:bc], Q[:, 0:bc], op=ALU.subtract
            )

            # det = r0 . cross(r1, r2);  rinv = 1 / (scale * det)
            T3 = hdr.tile([VPT, B, 4], F32, tag="T3")
            nc.vector.tensor_tensor(
                T3[:, 0:bc, 0:3], X[:, 0:bc, 0, :], hdrH[:, 0:bc, 1:4], op=ALU.mult
            )
            da = hdr.tile([VPT, B], F32, tag="da")
            nc.vector.tensor_tensor(
                da[:, 0:bc], T3[:, 0:bc, 0], T3[:, 0:bc, 1], op=ALU.add
            )
            det = hdr.tile([VPT, B], F32, tag="det")
            nc.vector.tensor_tensor(
                det[:, 0:bc], da[:, 0:bc], T3[:, 0:bc, 2], op=ALU.add
            )
            # u = -(det * scale): the whole weight set (M cols and biases)
            # is negated so the projection matmul computes -proj and pt can
            # be ADDED into the same PSUM bank via a second matmul,
            # yielding d = pt - proj with no DVE subtract.
            u = hdr.tile([VPT, B], F32, tag="u")
            nc.vector.scalar_tensor_tensor(
                u[:, 0:bc], det[:, 0:bc], -1.0, hdrH[:, 0:bc, 0],
                op0=ALU.mult, op1=ALU.mult,
            )
            rinv = hdr.tile([VPT, B], F32, tag="rinv")
            nc.vector.reciprocal(rinv[:, 0:bc], u[:, 0:bc])

            # M columns into hv (k 0..2 e=0, k 4..6 e=1): X * rinv
            hvk = hv.rearrange("p (g t k) -> p g t k", t=4, k=8)  # [128,4,4,8]
            hv5 = hvk[:, :, 0:3, :].rearrange("p g t (e c) -> p g t e c", e=2)
            W6 = hdr.tile([VPT, B, 2, 4], F32, tag="W6")
            if ngf > 0:
                hvM = hv5[:, 0:ngf, :, :, 0:3]
                X5 = X[:, 0:nf3].rearrange("p (g t) e c -> p g t e c", t=GR)
                r5 = (
                    rinv[:, 0:nf3]
                    .rearrange("p (g t) -> p g t", t=GR)
                    .unsqueeze(3).unsqueeze(4)
                    .broadcast_to([VPT, ngf, GR, 2, 3])
                )
                nc.vector.tensor_tensor(hvM, X5, r5, op=ALU.mult)
                # bias pre-sums: W6 = Mcol * t  (summed to c_e below)
                t5 = (
                    hdrH[:, 0:nf3, 10:13]
                    .rearrange("p (g t) c -> p g t c", t=GR)
                    .unsqueeze(3)
                    .broadcast_to([VPT, ngf, GR, 2, 3])
                )
                nc.vector.tensor_tensor(
                    W6[:, 0:nf3, :, 0:3].rearrange("p (g t) e c -> p g t e c", t=GR),
                    hvM, t5, op=ALU.mult,
                )
            if gs_t > 0:
                hvMt = hv5[:, ngf, 0:gs_t, :, 0:3]
                rt = (
                    rinv[:, nf3:bc].unsqueeze(2).unsqueeze(3)
                    .broadcast_to([VPT, gs_t, 2, 3])
                )
                nc.vector.tensor_tensor(hvMt, X[:, nf3:bc], rt, op=ALU.mult)
                tt5 = (
                    hdrH[:, nf3:bc, 10:13].unsqueeze(2)
                    .broadcast_to([VPT, gs_t, 2, 3])
                )
                nc.vector.tensor_tensor(W6[:, nf3:bc, :, 0:3], hvMt, tt5, op=ALU.mult)

            # bias c_e = sum_c W6[..., c], written straight into hv k=3,7
            cb = hdr.tile([VPT, B, 2], F32, tag="cb")
            nc.vector.tensor_tensor(
                cb[:, 0:bc], W6[:, 0:bc, :, 0], W6[:, 0:bc, :, 1], op=ALU.add
            )
            hvB5 = hvk[:, :, 0:3, :].rearrange("p g t (e c) -> p g t e c", e=2)
            if ngf > 0:
                nc.vector.tensor_tensor(
                    hvB5[:, 0:ngf, :, :, 3:4],
                    cb[:, 0:nf3]
                    .rearrange("p (g t) e -> p g t e", t=GR)
                    .unsqueeze(4),
                    W6[:, 0:nf3, :, 2:3]
                    .rearrange("p (g t) e c -> p g t e c", t=GR),
                    op=ALU.add,
                )
            if gs_t > 0:
                nc.vector.tensor_tensor(
                    hvB5[:, ngf, 0:gs_t, :, 3:4],
                    cb[:, nf3:bc].unsqueeze(3),
                    W6[:, nf3:bc, :, 2:3],
                    op=ALU.add,
                )

            # ---- transpose all weights for this chunk in one shot ----
            # the PSUM->SBUF copy also converts to bf16: the projection
            # matmul then runs at 1 cycle/row (vs 4 for fp32)
            F = 32 * ng
            tps = psum_t.tile([128, 128], F32, tag="tps")
            nc.tensor.transpose(tps[0:F, :], hv[:, 0:F], identity)
            lhsT = lhsp.tile([128, 128], BF16, tag="lhsT")
            nc.scalar.copy(lhsT[0:F, :], tps[0:F, :])

            # ---- per-group main compute (software-pipelined) ----
            # per group g: PE computes -proj (bf16), DVE adds pt (d), ACT
            # squares, DVE masks/sums, ACT sqrt+NUM-accum one group later
            # (so no engine queue head waits on a just-issued producer).
            pend = None  # (msq, w0, gs) awaiting sqrt + NUM accumulation
            for g in range(ng):
                lw0 = g * GR
                gs = min(GR, bc - lw0)
                w0 = t0 + lw0
                K = 8 * gs
                rb = 32 * g

                ptv = slab[:, lw0 : lw0 + gs, :].rearrange(
                    "p j (l c) -> p j c l", c=ROWW
                )
                proj = psum_p.tile([VPT, GR, 2, NPTS], F32, tag="proj")
                nc.tensor.matmul(
                    proj[:, 0:gs],
                    lhsT[rb : rb + K, :],
                    xq[rb : rb + K, 0 : gs * 2 * NPTS],
                    start=True,
                    stop=True,
                    tile_position=(rb, 0),
                )

                # d = pt + (-proj)
                d = work.tile([VPT, GR, 2, NPTS], F32, tag="d")
                nc.vector.tensor_tensor(
                    d[:, 0:gs], ptv[:, :, 0:2, :], proj[:, 0:gs], op=ALU.add
                )
                sq = work.tile([VPT, GR, 2, NPTS], F32, tag="sq")
                nc.scalar.activation(sq[:, 0:gs], d[:, 0:gs], AF.Square)

                # mask ops depend only on the slab: keep DVE busy while ACT
                # squares
                m = work.tile([VPT, GR, NPTS], F32, tag="m")
                nc.vector.tensor_tensor(
                    m[:, 0:gs], ptv[:, :, 0, :], ptv[:, :, 1, :], op=ALU.max
                )
                mge = work.tile([VPT, GR, NPTS], F32, tag="mge")
                for j in range(gs):
                    w = w0 + j
                    nc.vector.tensor_scalar(
                        mge[:, j], m[:, j], 0.0, None, op0=ALU.is_ge, op1=ALU.add,
                        accum_out=DEN[:, w : w + 1],
                    )
                ss = work.tile([VPT, GR, NPTS], F32, tag="ss")
                nc.vector.tensor_tensor(
                    ss[:, 0:gs], sq[:, 0:gs, 0, :], sq[:, 0:gs, 1, :], op=ALU.add
                )
                msq = work.tile([VPT, GR, NPTS], F32, tag="msq")
                nc.vector.tensor_tensor(
                    msq[:, 0:gs], ss[:, 0:gs], mge[:, 0:gs], op=ALU.mult
                )

                if pend is not None:
                    pmsq, pw0, pgs = pend
                    dist = work.tile([VPT, GR, NPTS], F32, tag="dist")
                    for j in range(pgs):
                        w = pw0 + j
                        nc.scalar.activation(
                            dist[:, j], pmsq[:, j], AF.Sqrt,
                            accum_out=NUM[:, w : w + 1],
                        )
                pend = (msq, w0, gs)
            if pend is not None:
                pmsq, pw0, pgs = pend
                dist = work.tile([VPT, GR, NPTS], F32, tag="dist")
                for j in range(pgs):
                    w = pw0 + j
                    nc.scalar.activation(
                        dist[:, j], pmsq[:, j], AF.Sqrt,
                        accum_out=NUM[:, w : w + 1],
                    )
            t0 += bc

        nc.sync.dma_start(out=num_o[:, :], in_=NUM)
        nc.sync.dma_start(out=den_o[:, :], in_=DEN)

    nc.compile()
    return nc, nt


_CACHE = {}


def _get_nc(v_core=V_CORE):
    key = v_core
    if key not in _CACHE:
        _CACHE[key] = build_nc(v_core)
    return _CACHE[key]


def make_xaug3(points_x):
    """Host-built [128, 408] streamed constant: block-diag [24, 408] of
    three [8, 136] xaug blocks, replicated in all 4 partition quadrants."""
    xa = np.zeros((8, 2 * NPTS), dtype=np.float32)
    xa[0:3, 0:NPTS] = points_x.T
    xa[3, 0:NPTS] = -1.0
    xa[4:7, NPTS:] = points_x.T
    xa[7, NPTS:] = -1.0
    import ml_dtypes
    xq = np.zeros((128, GR * 2 * NPTS), dtype=np.float32)
    for q in range(4):
        for b in range(GR):
            xq[32 * q + 8 * b : 32 * q + 8 * b + 8,
               2 * NPTS * b : 2 * NPTS * (b + 1)] = xa
    return xq.astype(ml_dtypes.bfloat16)


def host_finish(nums, dens, v_core, v_total):
    """Combine per-core [128, nt] num/den partials into the scalar loss."""
    total = 0.0
    for num, den in zip(nums, dens):
        nt = num.shape[1]
        lv = num.astype(np.float64) / den.astype(np.float64)
        for w in range(nt):
            valid = min(VPT, v_core - w * VPT)
            total += lv[:valid, w].sum()
    return np.float32(total / (float(v_total) * float(v_total)))


def build_in_maps(points_x, points_y, v_core):
    body = points_y[NPTS:]
    xq = make_xaug3(points_x)
    in_maps = []
    for c in range(N_CORES):
        shard = np.ascontiguousarray(
            body[c * v_core * NPTS : (c + 1) * v_core * NPTS]
        )
        in_maps.append({"y": shard, "xaug3": xq})
    return in_maps


def kernel(points_x, points_y):
    points_x = np.asarray(points_x, dtype=np.float32)
    points_y = np.asarray(points_y, dtype=np.float32)
    v_total = (points_y.shape[0] - NPTS) // NPTS
    v_core = v_total // N_CORES
    nc, nt = _get_nc(v_core)

    in_maps = build_in_maps(points_x, points_y, v_core)
    res = run_bass_kernel_spmd(nc, in_maps, list(range(N_CORES)))
    nums = [res.results[c]["num"] for c in range(N_CORES)]
    dens = [res.results[c]["den"] for c in range(N_CORES)]
    return host_finish(nums, dens, v_core, v_total)


# revision 8
# speedup vs baseline: 1.0879x; 1.0879x over previous
"""Trainium2 Bass kernel for nn_Loss2D_57432302682561 (v2).

Math per view v (V = 40000 views, 68 landmarks each):
    y block  = points_y[68 + 68v : 68 + 68(v+1)]          # [68, 15]
    pt       = y[:, 0:2]                                   # target 2D points
    scale    = y[0, 2];  R = y[0, 3:12].reshape(3,3);  t = y[0, 12:15]
    M        = inv(scale * R) = adj(R) / (scale * det(R))  # [3, 3]
    proj     = (points_x - t) @ M  -> first 2 cols         # [68, 2]
    mask     = (pt[:,0] >= 0) | (pt[:,1] >= 0)
    dist     = sqrt(sum((pt - proj)^2, -1))
    loss_v   = sum(dist * mask) / sum(mask)
    out      = sum_v loss_v / V^2

v2 design (8 NeuronCores, data-parallel over views; DMA-roofline bound):
  - Chunks of 12 tiles (128 views each) double-buffered; one contiguous
    slab DMA per chunk (full-bandwidth 4080B descriptors).
  - Header math (3x3 inverse) via duplicated-row cross products:
    inv(R) cols 0/1 = cross(r1,r2), cross(r2,r0) over scale*det.  ~20 wide
    ops per chunk instead of 41 tiny ops (dup-copies on ACT, math on DVE).
  - Weights for ALL tiles of a chunk transposed in ONE PE transpose
    ([128 views, 128 weight-cols] -> PSUM), one PSUM->SBUF copy.  Weight
    cols are grouped 32 per 3-tile group (24 data + 8 pad) so matmul
    lhsT slices land on legal partition bases {0,32,64,96}.
  - Projection: one PE matmul per 3-tile group with a block-diagonal
    [24, 408] streamed matrix (replicated in all 4 SBUF quadrants so
    lhsT/rhs partition bases match).
  - Elementwise work grouped over 3 tiles per op (DVE); Square on ACT;
    per-view mask count via fused tensor_scalar accum (cheap DVE
    accumulator); per-view distance sum via per-tile ACT Sqrt accum.
  - Per-core output: num[128, nt] (masked dist sum), den[128, nt]
    (mask count); host computes num/den and the final reduction.
"""

import os
import sys

import numpy as np

for _p in ("/opt/trn_rl_repo",):
    if _p not in sys.path and os.path.isdir(_p):
        sys.path.insert(0, _p)

import concourse.bass as bass
import concourse.bacc as bacc
import concourse.tile as tile
from concourse import mybir
from concourse.bass_utils import run_bass_kernel_spmd
from concourse.masks import make_identity
from contextlib import ExitStack

F32 = mybir.dt.float32
BF16 = mybir.dt.bfloat16
NPTS = 68
ROWW = 15
VROW = NPTS * ROWW  # 1020 floats per view block
N_CORES = 8
V_TOTAL = 40000
V_CORE = V_TOTAL // N_CORES  # 5000
VPT = 128  # views per tile (partition dim)
B = 9      # tiles per chunk
GR = 3     # tiles per matmul group


def build_nc(v_core=V_CORE):
    nt = (v_core + VPT - 1) // VPT
    chunk_sizes = []
    t = 0
    while t < nt:
        sz = min(B, nt - t)
        chunk_sizes.append(sz)
        t += sz

    nc = bacc.Bacc()
    y = nc.dram_tensor("y", [v_core * NPTS, ROWW], F32, kind="ExternalInput")
    xq_d = nc.dram_tensor("xaug3", [128, GR * 2 * NPTS], BF16, kind="ExternalInput")
    num_o = nc.dram_tensor("num", [VPT, nt], F32, kind="ExternalOutput")
    den_o = nc.dram_tensor("den", [VPT, nt], F32, kind="ExternalOutput")

    # [v, (l c)] view of the input: one view block = 1020 contiguous floats
    y2 = y.rearrange("(v l) c -> v (l c)", l=NPTS)

    AF = mybir.ActivationFunctionType
    ALU = mybir.AluOpType

    with ExitStack() as ctx:
        tc = ctx.enter_context(tile.TileContext(nc))
        consts = ctx.enter_context(tc.tile_pool(name="consts", bufs=1))
        slabs = ctx.enter_context(tc.tile_pool(name="slabs", bufs=3))
        hdr = ctx.enter_context(tc.tile_pool(name="hdr", bufs=2))
        hvp = ctx.enter_context(tc.tile_pool(name="hvp", bufs=2))
        lhsp = ctx.enter_context(tc.tile_pool(name="lhsp", bufs=2))
        work = ctx.enter_context(tc.tile_pool(name="work", bufs=3))
        outp = ctx.enter_context(tc.tile_pool(name="outp", bufs=1))
        psum_p = ctx.enter_context(tc.tile_pool(name="psum_p", bufs=4, space="PSUM"))
        psum_t = ctx.enter_context(tc.tile_pool(name="psum_t", bufs=2, space="PSUM"))

        identity = consts.tile([128, 128], F32)
        make_identity(nc, identity)

        # Streamed projection matrix: block-diagonal [24, 408] (three [8,136]
        # xaug blocks), replicated in all four partition quadrants so the
        # matmul rhs partition base can match any lhsT group base.
        xq = consts.tile([128, GR * 2 * NPTS], BF16, name="xq")
        nc.sync.dma_start(out=xq, in_=xq_d[:, :])

        NUM = outp.tile([VPT, nt], F32)
        DEN = outp.tile([VPT, nt], F32)

        t0 = 0
        for bc in chunk_sizes:
            v0 = t0 * VPT
            n_views = min(v_core - v0, bc * VPT)
            nf = n_views // VPT  # full tiles
            rem = n_views - nf * VPT
            ng = (bc + GR - 1) // GR          # matmul groups this chunk
            ngf = bc // GR                    # full (3-tile) groups
            nf3 = ngf * GR                    # tiles covered by full groups
            gs_t = bc - nf3                   # tail group size (0..2)

            # Small early header DMA: 13 floats per view (scale, R, t).
            # Issued on the GpSimd SWDGE queue: its ~1.5k tiny descriptors
            # would serialize the SP ring behind the previous chunk's bulk;
            # on the idle GpSimd queue they land during the previous
            # chunk's stream, so header math fully overlaps.
            hdrH = hdr.tile([VPT, B, 13], F32, tag="hdrH")
            if nf > 0:
                hsrc = y2[v0 : v0 + nf * VPT].rearrange(
                    "(w p) f -> p w f", p=VPT
                )[:, :, 2:15]
                nc.gpsimd.dma_start(out=hdrH[:, 0:nf, :], in_=hsrc)
            if rem > 0:
                nc.gpsimd.dma_start(
                    out=hdrH[0:rem, nf, :],
                    in_=y2[v0 + nf * VPT : v0 + n_views, 2:15],
                )
                # invalid partitions: real (replicated) headers keep all
                # lanes finite; host ignores their num/den entries
                nc.gpsimd.dma_start(
                    out=hdrH[rem:VPT, nf, :], in_=y2[v0 : v0 + VPT - rem, 2:15]
                )

            # Bulk stream, one DMA per 3-tile group so elementwise compute
            # starts as soon as each group lands.  The remainder tile's
            # partial/fill DMAs go FIRST: dependency tracking is
            # tile-granular, so emitting them after the bulk would order
            # them behind the whole chunk's compute reads.
            slab = slabs.tile([VPT, B, VROW], F32, tag="slab")
            if rem > 0:
                nc.sync.dma_start(
                    out=slab[0:rem, nf, :],
                    in_=y2[v0 + nf * VPT : v0 + n_views],
                )
                nc.sync.dma_start(
                    out=slab[rem:VPT, nf, :], in_=y2[v0 : v0 + VPT - rem]
                )
            for g in range(ng):
                lw0 = g * GR
                gv0 = v0 + lw0 * VPT
                gnf = min(max(nf - lw0, 0), GR)
                if gnf > 0:
                    src = y2[gv0 : gv0 + gnf * VPT].rearrange(
                        "(w p) f -> p w f", p=VPT
                    )
                    nc.sync.dma_start(
                        out=slab[:, lw0 : lw0 + gnf, :], in_=src
                    )

            # ---- header math: M = inv(scale*R) cols 0,1 + bias rows ----
            # hv[:, 32g + 8t + k], k in 0..7: [M00,M10,M20,c0,M01,M11,M21,c1]
            hv = hvp.tile([VPT, 128], F32, tag="hv")
            nc.gpsimd.memset(hv, 0.0)

            # duplicated row buffers for cross products:
            # A = [r1 r1-dup | r2 r2-dup], B = [r2 r2-dup | r0 r0-dup]
            A = hdr.tile([VPT, B, 2, 5], F32, tag="A")
            Bt = hdr.tile([VPT, B, 2, 5], F32, tag="Bt")
            r12 = hdrH[:, 0:bc, 4:10].rearrange("p w (r c) -> p w r c", r=2)
            nc.scalar.copy(A[:, 0:bc, :, 0:3], r12)
            nc.scalar.copy(A[:, 0:bc, :, 3:5], r12[:, :, :, 0:2])
            nc.scalar.copy(Bt[:, 0:bc, 0, 0:3], hdrH[:, 0:bc, 7:10])
            nc.scalar.copy(Bt[:, 0:bc, 0, 3:5], hdrH[:, 0:bc, 7:9])
            nc.scalar.copy(Bt[:, 0:bc, 1, 0:3], hdrH[:, 0:bc, 1:4])
            nc.scalar.copy(Bt[:, 0:bc, 1, 3:5], hdrH[:, 0:bc, 1:3])

            # X[:, w, 0, :] = cross(r1, r2);  X[:, w, 1, :] = cross(r2, r0)
            P = hdr.tile([VPT, B, 2, 3], F32, tag="P")
            Q = hdr.tile([VPT, B, 2, 3], F32, tag="Q")
            X = hdr.tile([VPT, B, 2, 3], F32, tag="X")
            nc.vector.tensor_tensor(
                P[:, 0:bc], A[:, 0:bc, :, 1:4], Bt[:, 0:bc, :, 2:5], op=ALU.mult
            )
            nc.vector.tensor_tensor(
                Q[:, 0:bc], A[:, 0:bc, :, 2:5], Bt[:, 0:bc, :, 1:4], op=ALU.mult
            )
            nc.vector.tensor_tensor(
                X[:, 0:bc], P[:, 0:bc], Q[:, 0:bc], op=ALU.subtract
            )

            # det = r0 . cross(r1, r2);  rinv = 1 / (scale * det)
            T3 = hdr.tile([VPT, B, 4], F32, tag="T3")
            nc.vector.tensor_tensor(
                T3[:, 0:bc, 0:3], X[:, 0:bc, 0, :], hdrH[:, 0:bc, 1:4], op=ALU.mult
            )
            da = hdr.tile([VPT, B], F32, tag="da")
            nc.vector.tensor_tensor(
                da[:, 0:bc], T3[:, 0:bc, 0], T3[:, 0:bc, 1], op=ALU.add
            )
            det = hdr.tile([VPT, B], F32, tag="det")
            nc.vector.tensor_tensor(
                det[:, 0:bc], da[:, 0:bc], T3[:, 0:bc, 2], op=ALU.add
            )
            # u = -(det * scale): the whole weight set (M cols and biases)
            # is negated so the projection matmul computes -proj and pt can
            # be ADDED into the same PSUM bank via a second matmul,
            # yielding d = pt - proj with no DVE subtract.
            u = hdr.tile([VPT, B], F32, tag="u")
            nc.vector.scalar_tensor_tensor(
                u[:, 0:bc], det[:, 0:bc], -1.0, hdrH[:, 0:bc, 0],
                op0=ALU.mult, op1=ALU.mult,
            )
            rinv = hdr.tile([VPT, B], F32, tag="rinv")
            nc.vector.reciprocal(rinv[:, 0:bc], u[:, 0:bc])

            # M columns into hv (k 0..2 e=0, k 4..6 e=1): X * rinv
            hvk = hv.rearrange("p (g t k) -> p g t k", t=4, k=8)  # [128,4,4,8]
            hv5 = hvk[:, :, 0:3, :].rearrange("p g t (e c) -> p g t e c", e=2)
            W6 = hdr.tile([VPT, B, 2, 4], F32, tag="W6")
            if ngf > 0:
                hvM = hv5[:, 0:ngf, :, :, 0:3]
                X5 = X[:, 0:nf3].rearrange("p (g t) e c -> p g t e c", t=GR)
                r5 = (
                    rinv[:, 0:nf3]
                    .rearrange("p (g t) -> p g t", t=GR)
                    .unsqueeze(3).unsqueeze(4)
                    .broadcast_to([VPT, ngf, GR, 2, 3])
                )
                nc.vector.tensor_tensor(hvM, X5, r5, op=ALU.mult)
                # bias pre-sums: W6 = Mcol * t  (summed to c_e below)
                t5 = (
                    hdrH[:, 0:nf3, 10:13]
                    .rearrange("p (g t) c -> p g t c", t=GR)
                    .unsqueeze(3)
                    .broadcast_to([VPT, ngf, GR, 2, 3])
                )
                nc.vector.tensor_tensor(
                    W6[:, 0:nf3, :, 0:3].rearrange("p (g t) e c -> p g t e c", t=GR),
                    hvM, t5, op=ALU.mult,
                )
            if gs_t > 0:
                hvMt = hv5[:, ngf, 0:gs_t, :, 0:3]
                rt = (
                    rinv[:, nf3:bc].unsqueeze(2).unsqueeze(3)
                    .broadcast_to([VPT, gs_t, 2, 3])
                )
                nc.vector.tensor_tensor(hvMt, X[:, nf3:bc], rt, op=ALU.mult)
                tt5 = (
                    hdrH[:, nf3:bc, 10:13].unsqueeze(2)
                    .broadcast_to([VPT, gs_t, 2, 3])
                )
                nc.vector.tensor_tensor(W6[:, nf3:bc, :, 0:3], hvMt, tt5, op=ALU.mult)

            # bias c_e = sum_c W6[..., c], written straight into hv k=3,7
            cb = hdr.tile([VPT, B, 2], F32, tag="cb")
            nc.vector.tensor_tensor(
                cb[:, 0:bc], W6[:, 0:bc, :, 0], W6[:, 0:bc, :, 1], op=ALU.add
            )
            hvB5 = hvk[:, :, 0:3, :].rearrange("p g t (e c) -> p g t e c", e=2)
            if ngf > 0:
                nc.vector.tensor_tensor(
                    hvB5[:, 0:ngf, :, :, 3:4],
                    cb[:, 0:nf3]
                    .rearrange("p (g t) e -> p g t e", t=GR)
                    .unsqueeze(4),
                    W6[:, 0:nf3, :, 2:3]
                    .rearrange("p (g t) e c -> p g t e c", t=GR),
                    op=ALU.add,
                )
            if gs_t > 0:
                nc.vector.tensor_tensor(
                    hvB5[:, ngf, 0:gs_t, :, 3:4],
                    cb[:, nf3:bc].unsqueeze(3),
                    W6[:, nf3:bc, :, 2:3],
                    op=ALU.add,
                )

            # ---- transpose all weights for this chunk in one shot ----
            # the PSUM->SBUF copy also converts to bf16: the projection
            # matmul then runs at 1 cycle/row (vs 4 for fp32)
            F = 32 * ng
            tps = psum_t.tile([128, 128], F32, tag="tps")
            nc.tensor.transpose(tps[0:F, :], hv[:, 0:F], identity)
            lhsT = lhsp.tile([128, 128], BF16, tag="lhsT")
            nc.scalar.copy(lhsT[0:F, :], tps[0:F, :])

            # ---- per-group main compute (software-pipelined) ----
            # per group g: PE computes -proj (bf16), DVE adds pt (d), ACT
            # squares, DVE masks/sums, ACT sqrt+NUM-accum one group later
            # (so no engine queue head waits on a just-issued producer).
            pend = None  # (msq, w0, gs) awaiting sqrt + NUM accumulation
            for g in range(ng):
                lw0 = g * GR
                gs = min(GR, bc - lw0)
                w0 = t0 + lw0
                K = 8 * gs
                rb = 32 * g

                ptv = slab[:, lw0 : lw0 + gs, :].rearrange(
                    "p j (l c) -> p j c l", c=ROWW
                )
                proj = psum_p.tile([VPT, GR, 2, NPTS], F32, tag="proj")
                nc.tensor.matmul(
                    proj[:, 0:gs],
                    lhsT[rb : rb + K, :],
                    xq[rb : rb + K, 0 : gs * 2 * NPTS],
                    start=True,
                    stop=True,
                    tile_position=(rb, 0),
                )

                # d = pt + (-proj)
                d = work.tile([VPT, GR, 2, NPTS], F32, tag="d")
                nc.vector.tensor_tensor(
                    d[:, 0:gs], ptv[:, :, 0:2, :], proj[:, 0:gs], op=ALU.add
                )
                sq = work.tile([VPT, GR, 2, NPTS], F32, tag="sq")
                nc.scalar.activation(sq[:, 0:gs], d[:, 0:gs], AF.Square)

                # mask ops depend only on the slab: keep DVE busy while ACT
                # squares
                m = work.tile([VPT, GR, NPTS], F32, tag="m")
                nc.vector.tensor_tensor(
                    m[:, 0:gs], ptv[:, :, 0, :], ptv[:, :, 1, :], op=ALU.max
                )
                mge = work.tile([VPT, GR, NPTS], F32, tag="mge")
                for j in range(gs):
                    w = w0 + j
                    nc.vector.tensor_scalar(
                        mge[:, j], m[:, j], 0.0, None, op0=ALU.is_ge, op1=ALU.add,
                        accum_out=DEN[:, w : w + 1],
                    )
                ss = work.tile([VPT, GR, NPTS], F32, tag="ss")
                nc.vector.tensor_tensor(
                    ss[:, 0:gs], sq[:, 0:gs, 0, :], sq[:, 0:gs, 1, :], op=ALU.add
                )
                msq = work.tile([VPT, GR, NPTS], F32, tag="msq")
                nc.vector.tensor_tensor(
                    msq[:, 0:gs], ss[:, 0:gs], mge[:, 0:gs], op=ALU.mult
                )

                if pend is not None:
                    pmsq, pw0, pgs = pend
                    dist = work.tile([VPT, GR, NPTS], F32, tag="dist")
                    for j in range(pgs):
                        w = pw0 + j
                        nc.scalar.activation(
                            dist[:, j], pmsq[:, j], AF.Sqrt,
                            accum_out=NUM[:, w : w + 1],
                        )
                pend = (msq, w0, gs)
            if pend is not None:
                pmsq, pw0, pgs = pend
                dist = work.tile([VPT, GR, NPTS], F32, tag="dist")
                for j in range(pgs):
                    w = pw0 + j
                    nc.scalar.activation(
                        dist[:, j], pmsq[:, j], AF.Sqrt,
                        accum_out=NUM[:, w : w + 1],
                    )
            t0 += bc

        nc.sync.dma_start(out=num_o[:, :], in_=NUM)
        nc.sync.dma_start(out=den_o[:, :], in_=DEN)

    nc.compile()
    return nc, nt


_CACHE = {}


def _get_nc(v_core=V_CORE):
    key = v_core
    if key not in _CACHE:
        _CACHE[key] = build_nc(v_core)
    return _CACHE[key]


def make_xaug3(points_x):
    """Host-built [128, 408] streamed constant: block-diag [24, 408] of
    three [8, 136] xaug blocks, replicated in all 4 partition quadrants."""
    xa = np.zeros((8, 2 * NPTS), dtype=np.float32)
    xa[0:3, 0:NPTS] = points_x.T
    xa[3, 0:NPTS] = -1.0
    xa[4:7, NPTS:] = points_x.T
    xa[7, NPTS:] = -1.0
    import ml_dtypes
    xq = np.zeros((128, GR * 2 * NPTS), dtype=np.float32)
    for q in range(4):
        for b in range(GR):
            xq[32 * q + 8 * b : 32 * q + 8 * b + 8,
               2 * NPTS * b : 2 * NPTS * (b + 1)] = xa
    return xq.astype(ml_dtypes.bfloat16)


def host_finish(nums, dens, v_core, v_total):
    """Combine per-core [128, nt] num/den partials into the scalar loss."""
    total = 0.0
    for num, den in zip(nums, dens):
        nt = num.shape[1]
        lv = num.astype(np.float64) / den.astype(np.float64)
        for w in range(nt):
            valid = min(VPT, v_core - w * VPT)
            total += lv[:valid, w].sum()
    return np.float32(total / (float(v_total) * float(v_total)))


def build_in_maps(points_x, points_y, v_core):
    body = points_y[NPTS:]
    xq = make_xaug3(points_x)
    in_maps = []
    for c in range(N_CORES):
        shard = np.ascontiguousarray(
            body[c * v_core * NPTS : (c + 1) * v_core * NPTS]
        )
        in_maps.append({"y": shard, "xaug3": xq})
    return in_maps


def kernel(points_x, points_y):
    points_x = np.asarray(points_x, dtype=np.float32)
    points_y = np.asarray(points_y, dtype=np.float32)
    v_total = (points_y.shape[0] - NPTS) // NPTS
    v_core = v_total // N_CORES
    nc, nt = _get_nc(v_core)

    in_maps = build_in_maps(points_x, points_y, v_core)
    res = run_bass_kernel_spmd(nc, in_maps, list(range(N_CORES)))
    nums = [res.results[c]["num"] for c in range(N_CORES)]
    dens = [res.results[c]["den"] for c in range(N_CORES)]
    return host_finish(nums, dens, v_core, v_total)
